# revision 32
# baseline (speedup 1.0000x reference)
"""Multi-head attention (B=4, S=2048, D=1024, H=16, Dh=64) on 8 TRN2 NeuronCores.

Sharding: core = (batch, head_group) with 4 batches x 2 head-groups of 8 heads.
Fully data-parallel SPMD - no collectives. Each core computes
out[b, :, hg*512:(hg+1)*512].

v2: all matmul operands in bf16 (inputs cast host-side, halving DMA and
PE passes vs fp32r), and the softmax denominators come for free from a
ones-column fused into each head's V stationary ([128, 65] per head ->
output row 64 accumulates sum(exp)).

Per-core kernel:
  phase 1: project K^T (d-major), V (k-major + ones col), Q^T.
  phase 2: per q-block of 512 and head-quad of 4: scores S^T[k,q] per
           k-tile via row-paired Dh=64 matmuls, exp on ScalarE
           (scale=1/8, per-partition bias -1e6 at the masked key
           position), AV via [128,65] stationaries accumulating
           [O^T; sums] in per-head PSUM tiles, then PE-transpose back
           to q-major and normalize on VectorE.
"""

from contextlib import ExitStack

import ml_dtypes
import numpy as np

import concourse.bass as bass
import concourse.bacc as bacc
import concourse.mybir as mybir
import concourse.tile as tile
from concourse.bass_utils import run_bass_kernel_spmd
from concourse.masks import make_identity

B = 4
SEQ = 2048
DM = 1024
H = 16
DH = 64
NCORES = 8
CPC = 512          # output columns per core (8 heads x 64)
P = 128
NQB = SEQ // 512   # q blocks of 512
NKT = SEQ // P     # k tiles of 128
NDT = DM // P      # d_model tiles of 128

F32 = mybir.dt.float32
BF16 = mybir.dt.bfloat16
EXP = mybir.ActivationFunctionType.Exp

_compiled = None


def _emit(ctx: ExitStack, tc: tile.TileContext, qt, kt, vt, wq, wk, wv, bmask, out):
    nc = tc.nc

    small = ctx.enter_context(tc.tile_pool(name="small", bufs=1))
    in_sb = ctx.enter_context(tc.tile_pool(name="in_sb", bufs=1))
    proj = ctx.enter_context(tc.tile_pool(name="proj", bufs=1))
    wpool = ctx.enter_context(tc.tile_pool(name="wpool", bufs=1))
    epool = ctx.enter_context(tc.tile_pool(name="epool", bufs=8))
    opool = ctx.enter_context(tc.tile_pool(name="opool", bufs=3))
    rpool = ctx.enter_context(tc.tile_pool(name="rpool", bufs=4))
    oparts = ctx.enter_context(tc.tile_pool(name="oparts", bufs=3))
    ps_sc = ctx.enter_context(tc.tile_pool(name="ps_sc", bufs=3, space="PSUM"))
    ps_ot = ctx.enter_context(tc.tile_pool(name="ps_ot", bufs=2, space="PSUM"))

    ident_f = small.tile([P, P], F32)
    make_identity(nc, ident_f[:])
    ident = small.tile([P, P], BF16)
    nc.vector.tensor_copy(ident[:], ident_f[:])
    bmask_sb = small.tile([P, NKT], F32)
    nc.sync.dma_start(bmask_sb[:], bmask.ap())

    # wk/wq are laid out [p, pe-col-block, dt, 128] so each projection
    # chunk p only needs a quarter of the tile; wv stays dt-major (its
    # matmuls stream all 512 columns per dt).
    w_sb = {
        "wk": wpool.tile([P, 4, NDT, 128], BF16, tag="wk", name="wk"),
        "wq": wpool.tile([P, 4, NDT, 128], BF16, tag="wq", name="wq"),
        "wv": wpool.tile([P, NDT, CPC], BF16, tag="wv", name="wv"),
    }

    kt_sb = in_sb.tile([P, NDT, SEQ], BF16, tag="kt_sb")
    vt_sb = in_sb.tile([P, NDT, SEQ], BF16, tag="vt_sb")
    qt_sb = in_sb.tile([P, NDT, SEQ], BF16, tag="qt_sb")

    # DMA issue order follows first-use order: weights, then per-512-block
    # K, V, Q column slices.
    # DMA issue order matches first-use order: the first pass consumes
    # K/V block 1 (rotated kt order) and Q block 0, then projects blocks
    # 2, 3, 0 from inside the pass.
    def in_dma(t, t_sb, blk):
        cs = slice(blk * 512, (blk + 1) * 512)
        nc.sync.dma_start(
            t_sb[:, :, cs],
            t.ap().rearrange("(dt p) q -> p dt q", p=P)[:, :, cs],
        )

    def in_dma_dt(t, t_sb, blk):
        cs = slice(blk * 512, (blk + 1) * 512)
        src = t.ap().rearrange("(dt p) q -> p dt q", p=P)
        for dt in range(NDT):
            nc.sync.dma_start(
                t_sb[:, dt:dt + 1, cs], src[:, dt:dt + 1, cs]
            )

    def w_dma_g(name, w, g):
        nc.sync.dma_start(
            w_sb[name][:, g],
            w.ap().rearrange("(dt p) (g c) -> p g dt c", p=P, g=4)[:, g],
        )

    w_dma_g("wk", wk, 0)
    in_dma_dt(kt, kt_sb, 1)
    w_dma_g("wq", wq, 0)
    in_dma_dt(qt, qt_sb, 0)
    nc.sync.dma_start(w_sb["wv"][:], wv.ap().rearrange("(dt p) c -> p dt c", p=P))
    in_dma_dt(vt, vt_sb, 1)
    for g in range(1, 4):
        w_dma_g("wk", wk, g)
        w_dma_g("wq", wq, g)
    for blk in (2, 3, 0):
        in_dma(kt, kt_sb, blk)
        in_dma(vt, vt_sb, blk)
    for blk in (1, 2, 3):
        in_dma(qt, qt_sb, blk)

    kproj = [proj.tile([P, SEQ], BF16, tag=f"kproj{p}", name=f"kproj{p}") for p in range(4)]
    qproj = [proj.tile([P, SEQ], BF16, tag=f"qproj{p}", name=f"qproj{p}") for p in range(4)]
    v_ones = proj.tile([P, NKT, 8, 65], BF16, tag="v_ones")
    nc.vector.memset(v_ones[:], 1.0)

    def proj_kq_chunk(name, src_sb, dsts, blk, p):
        cs = slice(blk * 512, (blk + 1) * 512)
        ps = ps_sc.tile([P, 1024], F32, tag="scores")
        for dt in range(NDT):
            nc.tensor.matmul(
                ps[:, 0:512],
                w_sb[name][:, p, dt, :],
                src_sb[:, dt, cs],
                start=(dt == 0),
                stop=(dt == NDT - 1),
            )
        nc.vector.tensor_copy(dsts[p][:, cs], ps[:, 0:512])

    def proj_v_chunk(kb, sub):
        kt_i = kb * 4 + sub
        ps = ps_sc.tile([P, 1024], F32, tag="scores")
        for dt in range(NDT):
            nc.tensor.matmul(
                ps[:, 0:512],
                vt_sb[:, dt, kt_i * P:(kt_i + 1) * P],
                w_sb["wv"][:, dt, :],
                start=(dt == 0),
                stop=(dt == NDT - 1),
            )
        nc.vector.tensor_copy(
            v_ones[:, kt_i, :, 0:64],
            ps[:, 0:512].rearrange("p (h c) -> p h c", c=64),
        )

    # ---- phase 1: projections needed before the first pass --------------
    # (kt iteration order is rotated so blocks 2, 3, 0 can be projected
    # while the first pass runs over block 1's k-tiles)
    KT_ORDER = [4, 5, 6, 7, 8, 9, 10, 11, 12, 13, 14, 15, 0, 1, 2, 3]
    proj_kq_chunk("wk", kt_sb, kproj, 1, 0)
    proj_kq_chunk("wq", qt_sb, qproj, 0, 0)

    # ---- phase 2: attention --------------------------------------------
    # Per quad, 32 (kt, pi) steps are software-pipelined: scores(s)+exp(s)
    # are emitted two steps ahead of AV(s), so the PE never waits on the
    # ScalarE exp. Remaining K/V projection chunks are interleaved into
    # the first quad, and Q(qb+1) into each qb's second quad, keeping
    # both engines continuously fed.
    for qb in range(NQB):
        qs = slice(qb * 512, (qb + 1) * 512)
        for pr in range(4):
            # fillers: (kind, blk, p) proj chunks emitted one list per step.
            # For the very first pass this covers everything beyond the two
            # upfront chunks; deadlines are met under the rotated KT_ORDER.
            filler = [[] for _ in range(16)]
            if qb == 0 and pr == 0:
                filler[0] += [("v", 1, 0), ("v", 1, 1)]
                filler[1] += [("v", 1, 2), ("v", 1, 3)]
                filler[2] += [("k", 1, 1)]
                filler[3] += [("k", 1, 2)]
                filler[4] += [("k", 1, 3)]
                filler[5] += [("q", 0, 1)]
                filler[6] += [("q", 0, 2)]
                filler[7] += [("q", 0, 3)]
                for j, kb in enumerate((2, 3, 0)):
                    for p in range(4):
                        filler[4 * j + p].append(("k", kb, p))
                        filler[4 * j + p].append(("v", kb, p))
            if pr == 3 and qb < NQB - 1:
                for p in range(4):
                    filler[4 * p + 1].append(("q", qb + 1, p))

            ot = [ps_ot.tile([P, 512], F32, tag="ot", name=f"ot{pr}_{hh}")
                  for hh in range(2)]
            e_tiles = {}

            def emit_scores(s):
                kt_i = KT_ORDER[s]
                sps = ps_sc.tile([P, 1024], F32, tag="scores")
                for hh in range(2):
                    rows = slice(64 * hh, 64 * (hh + 1))
                    nc.tensor.matmul(
                        sps[:, 512 * hh:512 * (hh + 1)],
                        kproj[pr][rows, kt_i * P:(kt_i + 1) * P],
                        qproj[pr][rows, qs],
                        start=True,
                        stop=True,
                        tile_position=(64 * hh, 0),
                    )
                e = epool.tile([P, 1024], BF16, tag="e")
                nc.scalar.activation(
                    e[:], sps[:], EXP,
                    bias=bmask_sb[:, kt_i:kt_i + 1], scale=0.125,
                )
                e_tiles[s] = e

            def emit_av(s):
                kt_i = KT_ORDER[s]
                e = e_tiles.pop(s)
                for hh in range(2):
                    h = 2 * pr + hh
                    nc.tensor.matmul(
                        ot[hh][0:65, :],
                        v_ones[:, kt_i, h, :],
                        e[:, 512 * hh:512 * (hh + 1)],
                        start=(s == 0),
                        stop=(s == NKT - 1),
                    )

            for s in range(16):
                emit_scores(s)
                for kind, blk, p in filler[s]:
                    if kind == "k":
                        proj_kq_chunk("wk", kt_sb, kproj, blk, p)
                    elif kind == "v":
                        proj_v_chunk(blk, p)
                    else:
                        proj_kq_chunk("wq", qt_sb, qproj, blk, p)
                if s >= 2:
                    emit_av(s - 2)
            emit_av(14)
            emit_av(15)

            # ---- tail: transpose + normalize + store -------------------
            o_part = oparts.tile([P, 4, 128], F32, tag="opart")
            for hh in range(2):
                ot_sb = opool.tile([P, 512], F32, tag="ot_sb")
                nc.vector.tensor_copy(ot_sb[0:65, :], ot[hh][0:65, :])
                tr = ps_ot.tile([P, 512], F32, tag="ot", name=f"tr{pr}_{hh}")
                rcp = rpool.tile([P, 4], F32, tag="rcp")
                for c in range(4):
                    nc.tensor.transpose(
                        tr[:, 65 * c:65 * c + 65],
                        ot_sb[0:65, c * P:(c + 1) * P],
                        ident_f[0:65, 0:65],
                    )
                for c in range(4):
                    nc.vector.reciprocal(
                        rcp[:, c:c + 1], tr[:, 65 * c + 64:65 * c + 65]
                    )
                for c in range(4):
                    nc.vector.tensor_scalar(
                        o_part[:, c, 64 * hh:64 * (hh + 1)],
                        tr[:, 65 * c:65 * c + 64],
                        rcp[:, c:c + 1],
                        None,
                        mybir.AluOpType.mult,
                    )
            nc.sync.dma_start(
                out.ap()[qb * 512:(qb + 1) * 512, pr * 128:(pr + 1) * 128]
                .rearrange("(c p) w -> p c w", p=P),
                o_part[:],
            )


def build():
    global _compiled
    if _compiled is not None:
        return _compiled
    nc = bacc.Bacc("TRN2", target_bir_lowering=False, debug=False)
    qt = nc.dram_tensor("qt", [DM, SEQ], BF16, kind="ExternalInput")
    kt = nc.dram_tensor("kt", [DM, SEQ], BF16, kind="ExternalInput")
    vt = nc.dram_tensor("vt", [DM, SEQ], BF16, kind="ExternalInput")
    wq = nc.dram_tensor("wq", [DM, CPC], BF16, kind="ExternalInput")
    wk = nc.dram_tensor("wk", [DM, CPC], BF16, kind="ExternalInput")
    wv = nc.dram_tensor("wv", [DM, CPC], BF16, kind="ExternalInput")
    bmask = nc.dram_tensor("bmask", [P, NKT], F32, kind="ExternalInput")
    out = nc.dram_tensor("out", [SEQ, CPC], F32, kind="ExternalOutput")
    with tile.TileContext(nc) as tc:
        with ExitStack() as ctx:
            _emit(ctx, tc, qt, kt, vt, wq, wk, wv, bmask, out)
    nc.compile()
    _compiled = nc
    return nc


def make_in_maps(Q_seq, K_seq, V_seq, V_len, WQ, WK, WV):
    bf = ml_dtypes.bfloat16
    in_maps = []
    qkv_t = {}
    for b in range(B):
        qkv_t[b] = tuple(
            np.ascontiguousarray(x[b].T).astype(bf) for x in (Q_seq, K_seq, V_seq)
        )
    w_bf = {hg: tuple(
        np.ascontiguousarray(w[:, hg * CPC:(hg + 1) * CPC]).astype(bf)
        for w in (WQ, WK, WV)) for hg in range(2)}
    for core in range(NCORES):
        b, hg = divmod(core, 2)
        bm = np.zeros((P, NKT), np.float32)
        vl = int(V_len[b, 0])
        bm[vl % P, vl // P] = -1e6
        qt, kt, vt = qkv_t[b]
        wq, wk, wv = w_bf[hg]
        in_maps.append(
            {"qt": qt, "kt": kt, "vt": vt, "wq": wq, "wk": wk, "wv": wv,
             "bmask": bm}
        )
    return in_maps


def kernel(Q_seq, K_seq, V_seq, Q_len, V_len, WQ, WK, WV, _trace=False):
    nc = build()
    in_maps = make_in_maps(Q_seq, K_seq, V_seq, V_len, WQ, WK, WV)
    res = run_bass_kernel_spmd(
        nc, in_maps, core_ids=list(range(NCORES)), trace=_trace
    )
    out = np.empty((B, SEQ, H * DH), np.float32)
    for core in range(NCORES):
        b, hg = divmod(core, 2)
        out[b, :, hg * CPC:(hg + 1) * CPC] = res.results[core]["out"]
    for b in range(B):
        out[b, int(Q_len[b, 0]), :] = 0.0
    if _trace:
        kernel._last_results = res
    return out


# revision 34
# speedup vs baseline: 1.1908x; 1.1908x over previous
"""Multi-head attention (B=4, S=2048, D=1024, H=16, Dh=64) on 8 TRN2 NeuronCores.

Sharding: core = (batch, head_group) with 4 batches x 2 head-groups of 8 heads.
Fully data-parallel SPMD - no collectives. Each core computes
out[b, :, hg*512:(hg+1)*512].

v2: all matmul operands in bf16 (inputs cast host-side, halving DMA and
PE passes vs fp32r), and the softmax denominators come for free from a
ones-column fused into each head's V stationary ([128, 65] per head ->
output row 64 accumulates sum(exp)).

Per-core kernel:
  phase 1: project K^T (d-major), V (k-major + ones col), Q^T.
  phase 2: per q-block of 512 and head-quad of 4: scores S^T[k,q] per
           k-tile via row-paired Dh=64 matmuls, exp on ScalarE
           (scale=1/8, per-partition bias -1e6 at the masked key
           position), AV via [128,65] stationaries accumulating
           [O^T; sums] in per-head PSUM tiles, then PE-transpose back
           to q-major and normalize on VectorE.
"""

from contextlib import ExitStack

import ml_dtypes
import numpy as np

import concourse.bass as bass
import concourse.bacc as bacc
import concourse.mybir as mybir
import concourse.tile as tile
from concourse.bass_utils import run_bass_kernel_spmd
from concourse.masks import make_identity

B = 4
SEQ = 2048
DM = 1024
H = 16
DH = 64
NCORES = 8
CPC = 512          # output columns per core (8 heads x 64)
P = 128
NQB = SEQ // 512   # q blocks of 512
NKT = SEQ // P     # k tiles of 128
NDT = DM // P      # d_model tiles of 128

F32 = mybir.dt.float32
BF16 = mybir.dt.bfloat16
EXP = mybir.ActivationFunctionType.Exp

_compiled = None


def _emit(ctx: ExitStack, tc: tile.TileContext, qt, kt, vt, wq, wk, wv, bmask, out):
    nc = tc.nc

    small = ctx.enter_context(tc.tile_pool(name="small", bufs=1))
    in_sb = ctx.enter_context(tc.tile_pool(name="in_sb", bufs=1))
    proj = ctx.enter_context(tc.tile_pool(name="proj", bufs=1))
    wpool = ctx.enter_context(tc.tile_pool(name="wpool", bufs=1))
    epool = ctx.enter_context(tc.tile_pool(name="epool", bufs=8))
    opool = ctx.enter_context(tc.tile_pool(name="opool", bufs=3))
    rpool = ctx.enter_context(tc.tile_pool(name="rpool", bufs=4))
    oparts = ctx.enter_context(tc.tile_pool(name="oparts", bufs=3))
    ps_sc = ctx.enter_context(tc.tile_pool(name="ps_sc", bufs=3, space="PSUM"))
    ps_ot = ctx.enter_context(tc.tile_pool(name="ps_ot", bufs=2, space="PSUM"))

    ident_f = small.tile([P, P], F32)
    make_identity(nc, ident_f[:])
    ident = small.tile([P, P], BF16)
    nc.vector.tensor_copy(ident[:], ident_f[:])
    bmask_sb = small.tile([P, NKT], F32)
    nc.sync.dma_start(bmask_sb[:], bmask.ap())

    # wk/wq are laid out [p, pe-col-block, dt, 128] so each projection
    # chunk p only needs a quarter of the tile; wv stays dt-major (its
    # matmuls stream all 512 columns per dt).
    w_sb = {
        "wk": wpool.tile([P, 4, NDT, 128], BF16, tag="wk", name="wk"),
        "wq": wpool.tile([P, 4, NDT, 128], BF16, tag="wq", name="wq"),
        "wv": wpool.tile([P, NDT, CPC], BF16, tag="wv", name="wv"),
    }

    kt_sb = in_sb.tile([P, NDT, SEQ], BF16, tag="kt_sb")
    vt_sb = in_sb.tile([P, NDT, SEQ], BF16, tag="vt_sb")
    qt_sb = in_sb.tile([P, NDT, SEQ], BF16, tag="qt_sb")

    # DMA issue order follows first-use order: weights, then per-512-block
    # K, V, Q column slices.
    # DMA issue order matches first-use order: the first pass consumes
    # K/V block 1 (rotated kt order) and Q block 0, then projects blocks
    # 2, 3, 0 from inside the pass.
    def in_dma(t, t_sb, blk):
        cs = slice(blk * 512, (blk + 1) * 512)
        nc.sync.dma_start(
            t_sb[:, :, cs],
            t.ap().rearrange("(dt p) q -> p dt q", p=P)[:, :, cs],
        )

    def in_dma_dt(t, t_sb, blk):
        cs = slice(blk * 512, (blk + 1) * 512)
        src = t.ap().rearrange("(dt p) q -> p dt q", p=P)
        for dt in range(NDT):
            nc.sync.dma_start(
                t_sb[:, dt:dt + 1, cs], src[:, dt:dt + 1, cs]
            )

    def w_dma_g(name, w, g):
        nc.sync.dma_start(
            w_sb[name][:, g],
            w.ap().rearrange("(dt p) (g c) -> p g dt c", p=P, g=4)[:, g],
        )

    w_dma_g("wk", wk, 0)
    in_dma_dt(kt, kt_sb, 1)
    w_dma_g("wq", wq, 0)
    in_dma_dt(qt, qt_sb, 0)
    nc.sync.dma_start(w_sb["wv"][:], wv.ap().rearrange("(dt p) c -> p dt c", p=P))
    in_dma_dt(vt, vt_sb, 1)
    for g in range(1, 4):
        w_dma_g("wk", wk, g)
        w_dma_g("wq", wq, g)
    for blk in (2, 3, 0):
        in_dma(kt, kt_sb, blk)
        in_dma(vt, vt_sb, blk)
    for blk in (1, 2, 3):
        in_dma(qt, qt_sb, blk)

    kproj = [proj.tile([P, SEQ], BF16, tag=f"kproj{p}", name=f"kproj{p}") for p in range(4)]
    qproj = [proj.tile([P, SEQ], BF16, tag=f"qproj{p}", name=f"qproj{p}") for p in range(4)]
    v_ones = proj.tile([P, NKT, 8, 65], BF16, tag="v_ones")
    nc.vector.memset(v_ones[:], 1.0)

    def proj_kq_chunk(name, src_sb, dsts, blk, p):
        cs = slice(blk * 512, (blk + 1) * 512)
        ps = ps_sc.tile([P, 1024], F32, tag="scores")
        for dt in range(NDT):
            nc.tensor.matmul(
                ps[:, 0:512],
                w_sb[name][:, p, dt, :],
                src_sb[:, dt, cs],
                start=(dt == 0),
                stop=(dt == NDT - 1),
            )
        nc.vector.tensor_copy(dsts[p][:, cs], ps[:, 0:512])

    def proj_v_chunk(kb, sub):
        kt_i = kb * 4 + sub
        ps = ps_sc.tile([P, 1024], F32, tag="scores")
        for dt in range(NDT):
            nc.tensor.matmul(
                ps[:, 0:512],
                vt_sb[:, dt, kt_i * P:(kt_i + 1) * P],
                w_sb["wv"][:, dt, :],
                start=(dt == 0),
                stop=(dt == NDT - 1),
            )
        nc.vector.tensor_copy(
            v_ones[:, kt_i, :, 0:64],
            ps[:, 0:512].rearrange("p (h c) -> p h c", c=64),
        )

    # ---- phase 1: projections needed before the first pass --------------
    # (kt iteration order is rotated so blocks 2, 3, 0 can be projected
    # while the first pass runs over block 1's k-tiles)
    KT_ORDER = [4, 5, 6, 7, 8, 9, 10, 11, 12, 13, 14, 15, 0, 1, 2, 3]
    for p in range(4):
        proj_kq_chunk("wk", kt_sb, kproj, 1, p)
    for sub in range(4):
        proj_v_chunk(1, sub)
    for p in range(4):
        proj_kq_chunk("wq", qt_sb, qproj, 0, p)

    # ---- phase 2: attention --------------------------------------------
    # Per quad, 32 (kt, pi) steps are software-pipelined: scores(s)+exp(s)
    # are emitted two steps ahead of AV(s), so the PE never waits on the
    # ScalarE exp. Remaining K/V projection chunks are interleaved into
    # the first quad, and Q(qb+1) into each qb's second quad, keeping
    # both engines continuously fed.
    for qb in range(NQB):
        qs = slice(qb * 512, (qb + 1) * 512)
        for pr in range(4):
            # fillers: (kind, blk, p) proj chunks emitted one list per step.
            # For the very first pass this covers everything beyond the two
            # upfront chunks; deadlines are met under the rotated KT_ORDER.
            filler = [[] for _ in range(16)]
            if qb == 0 and pr == 0:
                for j, kb in enumerate((2, 3, 0)):
                    for p in range(4):
                        filler[4 * j + p].append(("k", kb, p))
                        filler[4 * j + p].append(("v", kb, p))
            if pr == 3 and qb < NQB - 1:
                for p in range(4):
                    filler[4 * p + 1].append(("q", qb + 1, p))

            ot = [ps_ot.tile([P, 512], F32, tag="ot", name=f"ot{pr}_{hh}")
                  for hh in range(2)]
            e_tiles = {}

            def emit_scores(s):
                kt_i = KT_ORDER[s]
                sps = ps_sc.tile([P, 1024], F32, tag="scores")
                for hh in range(2):
                    rows = slice(64 * hh, 64 * (hh + 1))
                    nc.tensor.matmul(
                        sps[:, 512 * hh:512 * (hh + 1)],
                        kproj[pr][rows, kt_i * P:(kt_i + 1) * P],
                        qproj[pr][rows, qs],
                        start=True,
                        stop=True,
                        tile_position=(64 * hh, 0),
                    )
                e = epool.tile([P, 1024], BF16, tag="e")
                nc.scalar.activation(
                    e[:], sps[:], EXP,
                    bias=bmask_sb[:, kt_i:kt_i + 1], scale=0.125,
                )
                e_tiles[s] = e

            def emit_av(s):
                kt_i = KT_ORDER[s]
                e = e_tiles.pop(s)
                for hh in range(2):
                    h = 2 * pr + hh
                    nc.tensor.matmul(
                        ot[hh][0:65, :],
                        v_ones[:, kt_i, h, :],
                        e[:, 512 * hh:512 * (hh + 1)],
                        start=(s == 0),
                        stop=(s == NKT - 1),
                    )

            for s in range(16):
                emit_scores(s)
                for kind, blk, p in filler[s]:
                    if kind == "k":
                        proj_kq_chunk("wk", kt_sb, kproj, blk, p)
                    elif kind == "v":
                        proj_v_chunk(blk, p)
                    else:
                        proj_kq_chunk("wq", qt_sb, qproj, blk, p)
                if s >= 2:
                    emit_av(s - 2)
            emit_av(14)
            emit_av(15)

            # ---- tail: transpose + normalize + store -------------------
            o_part = oparts.tile([P, 4, 128], F32, tag="opart")
            for hh in range(2):
                ot_sb = opool.tile([P, 512], F32, tag="ot_sb")
                nc.vector.tensor_copy(ot_sb[0:65, :], ot[hh][0:65, :])
                tr = ps_ot.tile([P, 512], F32, tag="ot", name=f"tr{pr}_{hh}")
                rcp = rpool.tile([P, 4], F32, tag="rcp")
                for c in range(4):
                    nc.tensor.transpose(
                        tr[:, 65 * c:65 * c + 65],
                        ot_sb[0:65, c * P:(c + 1) * P],
                        ident_f[0:65, 0:65],
                    )
                for c in range(4):
                    nc.vector.reciprocal(
                        rcp[:, c:c + 1], tr[:, 65 * c + 64:65 * c + 65]
                    )
                for c in range(4):
                    nc.vector.tensor_scalar(
                        o_part[:, c, 64 * hh:64 * (hh + 1)],
                        tr[:, 65 * c:65 * c + 64],
                        rcp[:, c:c + 1],
                        None,
                        mybir.AluOpType.mult,
                    )
            nc.sync.dma_start(
                out.ap()[qb * 512:(qb + 1) * 512, pr * 128:(pr + 1) * 128]
                .rearrange("(c p) w -> p c w", p=P),
                o_part[:],
            )


def build():
    global _compiled
    if _compiled is not None:
        return _compiled
    nc = bacc.Bacc("TRN2", target_bir_lowering=False, debug=False)
    qt = nc.dram_tensor("qt", [DM, SEQ], BF16, kind="ExternalInput")
    kt = nc.dram_tensor("kt", [DM, SEQ], BF16, kind="ExternalInput")
    vt = nc.dram_tensor("vt", [DM, SEQ], BF16, kind="ExternalInput")
    wq = nc.dram_tensor("wq", [DM, CPC], BF16, kind="ExternalInput")
    wk = nc.dram_tensor("wk", [DM, CPC], BF16, kind="ExternalInput")
    wv = nc.dram_tensor("wv", [DM, CPC], BF16, kind="ExternalInput")
    bmask = nc.dram_tensor("bmask", [P, NKT], F32, kind="ExternalInput")
    out = nc.dram_tensor("out", [SEQ, CPC], F32, kind="ExternalOutput")
    with tile.TileContext(nc) as tc:
        with ExitStack() as ctx:
            _emit(ctx, tc, qt, kt, vt, wq, wk, wv, bmask, out)
    nc.compile()
    _compiled = nc
    return nc


def make_in_maps(Q_seq, K_seq, V_seq, V_len, WQ, WK, WV):
    bf = ml_dtypes.bfloat16
    in_maps = []
    qkv_t = {}
    for b in range(B):
        qkv_t[b] = tuple(
            np.ascontiguousarray(x[b].T).astype(bf) for x in (Q_seq, K_seq, V_seq)
        )
    w_bf = {hg: tuple(
        np.ascontiguousarray(w[:, hg * CPC:(hg + 1) * CPC]).astype(bf)
        for w in (WQ, WK, WV)) for hg in range(2)}
    for core in range(NCORES):
        b, hg = divmod(core, 2)
        bm = np.zeros((P, NKT), np.float32)
        vl = int(V_len[b, 0])
        bm[vl % P, vl // P] = -1e6
        qt, kt, vt = qkv_t[b]
        wq, wk, wv = w_bf[hg]
        in_maps.append(
            {"qt": qt, "kt": kt, "vt": vt, "wq": wq, "wk": wk, "wv": wv,
             "bmask": bm}
        )
    return in_maps


def kernel(Q_seq, K_seq, V_seq, Q_len, V_len, WQ, WK, WV, _trace=False):
    nc = build()
    in_maps = make_in_maps(Q_seq, K_seq, V_seq, V_len, WQ, WK, WV)
    res = run_bass_kernel_spmd(
        nc, in_maps, core_ids=list(range(NCORES)), trace=_trace
    )
    out = np.empty((B, SEQ, H * DH), np.float32)
    for core in range(NCORES):
        b, hg = divmod(core, 2)
        out[b, :, hg * CPC:(hg + 1) * CPC] = res.results[core]["out"]
    for b in range(B):
        out[b, int(Q_len[b, 0]), :] = 0.0
    if _trace:
        kernel._last_results = res
    return out


# revision 35
# speedup vs baseline: 1.2340x; 1.0363x over previous
"""Multi-head attention (B=4, S=2048, D=1024, H=16, Dh=64) on 8 TRN2 NeuronCores.

Sharding: core = (batch, head_group) with 4 batches x 2 head-groups of 8 heads.
Fully data-parallel SPMD - no collectives. Each core computes
out[b, :, hg*512:(hg+1)*512].

v2: all matmul operands in bf16 (inputs cast host-side, halving DMA and
PE passes vs fp32r), and the softmax denominators come for free from a
ones-column fused into each head's V stationary ([128, 65] per head ->
output row 64 accumulates sum(exp)).

Per-core kernel:
  phase 1: project K^T (d-major), V (k-major + ones col), Q^T.
  phase 2: per q-block of 512 and head-quad of 4: scores S^T[k,q] per
           k-tile via row-paired Dh=64 matmuls, exp on ScalarE
           (scale=1/8, per-partition bias -1e6 at the masked key
           position), AV via [128,65] stationaries accumulating
           [O^T; sums] in per-head PSUM tiles, then PE-transpose back
           to q-major and normalize on VectorE.
"""

from contextlib import ExitStack

import ml_dtypes
import numpy as np

import concourse.bass as bass
import concourse.bacc as bacc
import concourse.mybir as mybir
import concourse.tile as tile
from concourse.bass_utils import run_bass_kernel_spmd
from concourse.masks import make_identity

B = 4
SEQ = 2048
DM = 1024
H = 16
DH = 64
NCORES = 8
CPC = 512          # output columns per core (8 heads x 64)
P = 128
NQB = SEQ // 512   # q blocks of 512
NKT = SEQ // P     # k tiles of 128
NDT = DM // P      # d_model tiles of 128

F32 = mybir.dt.float32
BF16 = mybir.dt.bfloat16
EXP = mybir.ActivationFunctionType.Exp

_compiled = None


def _emit(ctx: ExitStack, tc: tile.TileContext, qt, kt, vt, wq, wk, wv, bmask, out):
    nc = tc.nc

    small = ctx.enter_context(tc.tile_pool(name="small", bufs=1))
    in_sb = ctx.enter_context(tc.tile_pool(name="in_sb", bufs=1))
    proj = ctx.enter_context(tc.tile_pool(name="proj", bufs=1))
    wpool = ctx.enter_context(tc.tile_pool(name="wpool", bufs=1))
    epool = ctx.enter_context(tc.tile_pool(name="epool", bufs=8))
    opool = ctx.enter_context(tc.tile_pool(name="opool", bufs=3))
    rpool = ctx.enter_context(tc.tile_pool(name="rpool", bufs=4))
    oparts = ctx.enter_context(tc.tile_pool(name="oparts", bufs=3))
    ps_sc = ctx.enter_context(tc.tile_pool(name="ps_sc", bufs=3, space="PSUM"))
    ps_ot = ctx.enter_context(tc.tile_pool(name="ps_ot", bufs=2, space="PSUM"))

    ident_f = small.tile([P, P], F32)
    make_identity(nc, ident_f[:])
    ident = small.tile([P, P], BF16)
    nc.vector.tensor_copy(ident[:], ident_f[:])
    bmask_sb = small.tile([P, NKT], F32)
    nc.sync.dma_start(bmask_sb[:], bmask.ap())

    # wk/wq are laid out [p, pe-col-block, dt, 128] so each projection
    # chunk p only needs a quarter of the tile; wv stays dt-major (its
    # matmuls stream all 512 columns per dt).
    w_sb = {
        "wk": wpool.tile([P, 4, NDT, 128], BF16, tag="wk", name="wk"),
        "wq": wpool.tile([P, 4, NDT, 128], BF16, tag="wq", name="wq"),
        "wv": wpool.tile([P, NDT, CPC], BF16, tag="wv", name="wv"),
    }

    kt_sb = in_sb.tile([P, NDT, SEQ], BF16, tag="kt_sb")
    vt_sb = in_sb.tile([P, NDT, SEQ], BF16, tag="vt_sb")
    qt_sb = in_sb.tile([P, NDT, SEQ], BF16, tag="qt_sb")

    # DMA issue order follows first-use order: weights, then per-512-block
    # K, V, Q column slices.
    # DMA issue order matches first-use order: the first pass consumes
    # K/V block 1 (rotated kt order) and Q block 0, then projects blocks
    # 2, 3, 0 from inside the pass.
    def in_dma(t, t_sb, blk):
        cs = slice(blk * 512, (blk + 1) * 512)
        nc.sync.dma_start(
            t_sb[:, :, cs],
            t.ap().rearrange("(dt p) q -> p dt q", p=P)[:, :, cs],
        )

    def in_dma_dt(t, t_sb, blk):
        cs = slice(blk * 512, (blk + 1) * 512)
        src = t.ap().rearrange("(dt p) q -> p dt q", p=P)
        for dt in range(NDT):
            nc.sync.dma_start(
                t_sb[:, dt:dt + 1, cs], src[:, dt:dt + 1, cs]
            )

    def w_dma_g(name, w, g):
        nc.sync.dma_start(
            w_sb[name][:, g],
            w.ap().rearrange("(dt p) (g c) -> p g dt c", p=P, g=4)[:, g],
        )

    w_dma_g("wk", wk, 0)
    in_dma_dt(kt, kt_sb, 1)
    for g in range(1, 4):
        w_dma_g("wk", wk, g)
    nc.sync.dma_start(w_sb["wv"][:], wv.ap().rearrange("(dt p) c -> p dt c", p=P))
    in_dma_dt(vt, vt_sb, 1)
    w_dma_g("wq", wq, 0)
    in_dma_dt(qt, qt_sb, 0)
    for g in range(1, 4):
        w_dma_g("wq", wq, g)
    for blk in (2, 3, 0):
        in_dma(kt, kt_sb, blk)
        in_dma(vt, vt_sb, blk)
    for blk in (1, 2, 3):
        in_dma(qt, qt_sb, blk)

    kproj = [proj.tile([P, SEQ], BF16, tag=f"kproj{p}", name=f"kproj{p}") for p in range(4)]
    qproj = [proj.tile([P, SEQ], BF16, tag=f"qproj{p}", name=f"qproj{p}") for p in range(4)]
    v_ones = proj.tile([P, NKT, 8, 65], BF16, tag="v_ones")
    nc.vector.memset(v_ones[:], 1.0)

    def proj_kq_chunk(name, src_sb, dsts, blk, p):
        cs = slice(blk * 512, (blk + 1) * 512)
        ps = ps_sc.tile([P, 1024], F32, tag="scores")
        for dt in range(NDT):
            nc.tensor.matmul(
                ps[:, 0:512],
                w_sb[name][:, p, dt, :],
                src_sb[:, dt, cs],
                start=(dt == 0),
                stop=(dt == NDT - 1),
            )
        nc.vector.tensor_copy(dsts[p][:, cs], ps[:, 0:512])

    def proj_v_chunk(kb, sub):
        kt_i = kb * 4 + sub
        ps = ps_sc.tile([P, 1024], F32, tag="scores")
        for dt in range(NDT):
            nc.tensor.matmul(
                ps[:, 0:512],
                vt_sb[:, dt, kt_i * P:(kt_i + 1) * P],
                w_sb["wv"][:, dt, :],
                start=(dt == 0),
                stop=(dt == NDT - 1),
            )
        nc.vector.tensor_copy(
            v_ones[:, kt_i, :, 0:64],
            ps[:, 0:512].rearrange("p (h c) -> p h c", c=64),
        )

    # ---- phase 1: projections needed before the first pass --------------
    # (kt iteration order is rotated so blocks 2, 3, 0 can be projected
    # while the first pass runs over block 1's k-tiles)
    KT_ORDER = [4, 5, 6, 7, 8, 9, 10, 11, 12, 13, 14, 15, 0, 1, 2, 3]
    for p in range(4):
        proj_kq_chunk("wk", kt_sb, kproj, 1, p)
    for sub in range(4):
        proj_v_chunk(1, sub)
    for p in range(4):
        proj_kq_chunk("wq", qt_sb, qproj, 0, p)

    # ---- phase 2: attention --------------------------------------------
    # Per quad, 32 (kt, pi) steps are software-pipelined: scores(s)+exp(s)
    # are emitted two steps ahead of AV(s), so the PE never waits on the
    # ScalarE exp. Remaining K/V projection chunks are interleaved into
    # the first quad, and Q(qb+1) into each qb's second quad, keeping
    # both engines continuously fed.
    for qb in range(NQB):
        qs = slice(qb * 512, (qb + 1) * 512)
        for pr in range(4):
            # fillers: (kind, blk, p) proj chunks emitted one list per step.
            # For the very first pass this covers everything beyond the two
            # upfront chunks; deadlines are met under the rotated KT_ORDER.
            filler = [[] for _ in range(16)]
            if qb == 0 and pr == 0:
                for j, kb in enumerate((2, 3, 0)):
                    for p in range(4):
                        filler[4 * j + p].append(("k", kb, p))
                        filler[4 * j + p].append(("v", kb, p))
            if pr == 3 and qb < NQB - 1:
                for p in range(4):
                    filler[4 * p + 1].append(("q", qb + 1, p))

            ot = [ps_ot.tile([P, 512], F32, tag="ot", name=f"ot{pr}_{hh}")
                  for hh in range(2)]
            e_tiles = {}

            def emit_scores(s):
                kt_i = KT_ORDER[s]
                sps = ps_sc.tile([P, 1024], F32, tag="scores")
                for hh in range(2):
                    rows = slice(64 * hh, 64 * (hh + 1))
                    nc.tensor.matmul(
                        sps[:, 512 * hh:512 * (hh + 1)],
                        kproj[pr][rows, kt_i * P:(kt_i + 1) * P],
                        qproj[pr][rows, qs],
                        start=True,
                        stop=True,
                        tile_position=(64 * hh, 0),
                    )
                e = epool.tile([P, 1024], BF16, tag="e")
                nc.scalar.activation(
                    e[:], sps[:], EXP,
                    bias=bmask_sb[:, kt_i:kt_i + 1], scale=0.125,
                )
                e_tiles[s] = e

            def emit_av(s):
                kt_i = KT_ORDER[s]
                e = e_tiles.pop(s)
                for hh in range(2):
                    h = 2 * pr + hh
                    nc.tensor.matmul(
                        ot[hh][0:65, :],
                        v_ones[:, kt_i, h, :],
                        e[:, 512 * hh:512 * (hh + 1)],
                        start=(s == 0),
                        stop=(s == NKT - 1),
                    )

            for s in range(16):
                emit_scores(s)
                for kind, blk, p in filler[s]:
                    if kind == "k":
                        proj_kq_chunk("wk", kt_sb, kproj, blk, p)
                    elif kind == "v":
                        proj_v_chunk(blk, p)
                    else:
                        proj_kq_chunk("wq", qt_sb, qproj, blk, p)
                if s >= 2:
                    emit_av(s - 2)
            emit_av(14)
            emit_av(15)

            # ---- tail: transpose + normalize + store -------------------
            o_part = oparts.tile([P, 4, 128], F32, tag="opart")
            for hh in range(2):
                ot_sb = opool.tile([P, 512], F32, tag="ot_sb")
                nc.vector.tensor_copy(ot_sb[0:65, :], ot[hh][0:65, :])
                tr = ps_ot.tile([P, 512], F32, tag="ot", name=f"tr{pr}_{hh}")
                rcp = rpool.tile([P, 4], F32, tag="rcp")
                for c in range(4):
                    nc.tensor.transpose(
                        tr[:, 65 * c:65 * c + 65],
                        ot_sb[0:65, c * P:(c + 1) * P],
                        ident_f[0:65, 0:65],
                    )
                for c in range(4):
                    nc.vector.reciprocal(
                        rcp[:, c:c + 1], tr[:, 65 * c + 64:65 * c + 65]
                    )
                for c in range(4):
                    nc.vector.tensor_scalar(
                        o_part[:, c, 64 * hh:64 * (hh + 1)],
                        tr[:, 65 * c:65 * c + 64],
                        rcp[:, c:c + 1],
                        None,
                        mybir.AluOpType.mult,
                    )
            nc.sync.dma_start(
                out.ap()[qb * 512:(qb + 1) * 512, pr * 128:(pr + 1) * 128]
                .rearrange("(c p) w -> p c w", p=P),
                o_part[:],
            )


def build():
    global _compiled
    if _compiled is not None:
        return _compiled
    nc = bacc.Bacc("TRN2", target_bir_lowering=False, debug=False)
    qt = nc.dram_tensor("qt", [DM, SEQ], BF16, kind="ExternalInput")
    kt = nc.dram_tensor("kt", [DM, SEQ], BF16, kind="ExternalInput")
    vt = nc.dram_tensor("vt", [DM, SEQ], BF16, kind="ExternalInput")
    wq = nc.dram_tensor("wq", [DM, CPC], BF16, kind="ExternalInput")
    wk = nc.dram_tensor("wk", [DM, CPC], BF16, kind="ExternalInput")
    wv = nc.dram_tensor("wv", [DM, CPC], BF16, kind="ExternalInput")
    bmask = nc.dram_tensor("bmask", [P, NKT], F32, kind="ExternalInput")
    out = nc.dram_tensor("out", [SEQ, CPC], F32, kind="ExternalOutput")
    with tile.TileContext(nc) as tc:
        with ExitStack() as ctx:
            _emit(ctx, tc, qt, kt, vt, wq, wk, wv, bmask, out)
    nc.compile()
    _compiled = nc
    return nc


def make_in_maps(Q_seq, K_seq, V_seq, V_len, WQ, WK, WV):
    bf = ml_dtypes.bfloat16
    in_maps = []
    qkv_t = {}
    for b in range(B):
        qkv_t[b] = tuple(
            np.ascontiguousarray(x[b].T).astype(bf) for x in (Q_seq, K_seq, V_seq)
        )
    w_bf = {hg: tuple(
        np.ascontiguousarray(w[:, hg * CPC:(hg + 1) * CPC]).astype(bf)
        for w in (WQ, WK, WV)) for hg in range(2)}
    for core in range(NCORES):
        b, hg = divmod(core, 2)
        bm = np.zeros((P, NKT), np.float32)
        vl = int(V_len[b, 0])
        bm[vl % P, vl // P] = -1e6
        qt, kt, vt = qkv_t[b]
        wq, wk, wv = w_bf[hg]
        in_maps.append(
            {"qt": qt, "kt": kt, "vt": vt, "wq": wq, "wk": wk, "wv": wv,
             "bmask": bm}
        )
    return in_maps


def kernel(Q_seq, K_seq, V_seq, Q_len, V_len, WQ, WK, WV, _trace=False):
    nc = build()
    in_maps = make_in_maps(Q_seq, K_seq, V_seq, V_len, WQ, WK, WV)
    res = run_bass_kernel_spmd(
        nc, in_maps, core_ids=list(range(NCORES)), trace=_trace
    )
    out = np.empty((B, SEQ, H * DH), np.float32)
    for core in range(NCORES):
        b, hg = divmod(core, 2)
        out[b, :, hg * CPC:(hg + 1) * CPC] = res.results[core]["out"]
    for b in range(B):
        out[b, int(Q_len[b, 0]), :] = 0.0
    if _trace:
        kernel._last_results = res
    return out


# revision 37
# speedup vs baseline: 1.2466x; 1.0102x over previous
"""Multi-head attention (B=4, S=2048, D=1024, H=16, Dh=64) on 8 TRN2 NeuronCores.

Sharding: core = (batch, head_group) with 4 batches x 2 head-groups of 8 heads.
Fully data-parallel SPMD - no collectives. Each core computes
out[b, :, hg*512:(hg+1)*512].

v2: all matmul operands in bf16 (inputs cast host-side, halving DMA and
PE passes vs fp32r), and the softmax denominators come for free from a
ones-column fused into each head's V stationary ([128, 65] per head ->
output row 64 accumulates sum(exp)).

Per-core kernel:
  phase 1: project K^T (d-major), V (k-major + ones col), Q^T.
  phase 2: per q-block of 512 and head-quad of 4: scores S^T[k,q] per
           k-tile via row-paired Dh=64 matmuls, exp on ScalarE
           (scale=1/8, per-partition bias -1e6 at the masked key
           position), AV via [128,65] stationaries accumulating
           [O^T; sums] in per-head PSUM tiles, then PE-transpose back
           to q-major and normalize on VectorE.
"""

from contextlib import ExitStack

import ml_dtypes
import numpy as np

import concourse.bass as bass
import concourse.bacc as bacc
import concourse.mybir as mybir
import concourse.tile as tile
from concourse.bass_utils import run_bass_kernel_spmd
from concourse.masks import make_identity

B = 4
SEQ = 2048
DM = 1024
H = 16
DH = 64
NCORES = 8
CPC = 512          # output columns per core (8 heads x 64)
P = 128
NQB = SEQ // 512   # q blocks of 512
NKT = SEQ // P     # k tiles of 128
NDT = DM // P      # d_model tiles of 128

F32 = mybir.dt.float32
BF16 = mybir.dt.bfloat16
EXP = mybir.ActivationFunctionType.Exp

_compiled = None


def _emit(ctx: ExitStack, tc: tile.TileContext, qt, kt, vt, wq, wk, wv, bmask, out):
    nc = tc.nc

    small = ctx.enter_context(tc.tile_pool(name="small", bufs=1))
    in_sb = ctx.enter_context(tc.tile_pool(name="in_sb", bufs=1))
    proj = ctx.enter_context(tc.tile_pool(name="proj", bufs=1))
    wpool = ctx.enter_context(tc.tile_pool(name="wpool", bufs=1))
    epool = ctx.enter_context(tc.tile_pool(name="epool", bufs=8))
    opool = ctx.enter_context(tc.tile_pool(name="opool", bufs=3))
    rpool = ctx.enter_context(tc.tile_pool(name="rpool", bufs=4))
    oparts = ctx.enter_context(tc.tile_pool(name="oparts", bufs=3))
    ps_sc = ctx.enter_context(tc.tile_pool(name="ps_sc", bufs=3, space="PSUM"))
    ps_ot = ctx.enter_context(tc.tile_pool(name="ps_ot", bufs=2, space="PSUM"))

    ident_f = small.tile([P, P], F32)
    make_identity(nc, ident_f[:])
    ident = small.tile([P, P], BF16)
    nc.vector.tensor_copy(ident[:], ident_f[:])
    bmask_sb = small.tile([P, NKT], F32)
    nc.sync.dma_start(bmask_sb[:], bmask.ap())

    # wk/wq are laid out [p, pe-col-block, dt, 128] so each projection
    # chunk p only needs a quarter of the tile; wv stays dt-major (its
    # matmuls stream all 512 columns per dt).
    w_sb = {
        "wk": wpool.tile([P, 4, NDT, 128], BF16, tag="wk", name="wk"),
        "wq": wpool.tile([P, 4, NDT, 128], BF16, tag="wq", name="wq"),
        "wv": wpool.tile([P, NDT, CPC], BF16, tag="wv", name="wv"),
    }

    kt_sb = in_sb.tile([P, NDT, SEQ], BF16, tag="kt_sb")
    vt_sb = in_sb.tile([P, NDT, SEQ], BF16, tag="vt_sb")
    qt_sb = in_sb.tile([P, NDT, SEQ], BF16, tag="qt_sb")

    # DMA issue order follows first-use order: weights, then per-512-block
    # K, V, Q column slices.
    # DMA issue order matches first-use order: the first pass consumes
    # K/V block 1 (rotated kt order) and Q block 0, then projects blocks
    # 2, 3, 0 from inside the pass.
    def in_dma(t, t_sb, blk):
        cs = slice(blk * 512, (blk + 1) * 512)
        nc.sync.dma_start(
            t_sb[:, :, cs],
            t.ap().rearrange("(dt p) q -> p dt q", p=P)[:, :, cs],
        )

    def in_dma_dt(t, t_sb, blk):
        cs = slice(blk * 512, (blk + 1) * 512)
        src = t.ap().rearrange("(dt p) q -> p dt q", p=P)
        for dt in range(NDT):
            nc.sync.dma_start(
                t_sb[:, dt:dt + 1, cs], src[:, dt:dt + 1, cs]
            )

    def w_dma_g(name, w, g):
        nc.sync.dma_start(
            w_sb[name][:, g],
            w.ap().rearrange("(dt p) (g c) -> p g dt c", p=P, g=4)[:, g],
        )

    w_dma_g("wk", wk, 0)
    in_dma_dt(kt, kt_sb, 1)
    for g in range(1, 4):
        w_dma_g("wk", wk, g)
    nc.sync.dma_start(w_sb["wv"][:], wv.ap().rearrange("(dt p) c -> p dt c", p=P))
    in_dma_dt(vt, vt_sb, 1)
    w_dma_g("wq", wq, 0)
    in_dma_dt(qt, qt_sb, 0)
    for g in range(1, 4):
        w_dma_g("wq", wq, g)
    for blk in (2, 3, 0):
        in_dma(kt, kt_sb, blk)
        in_dma(vt, vt_sb, blk)
    for blk in (1, 2, 3):
        in_dma(qt, qt_sb, blk)

    kproj = [proj.tile([P, SEQ], BF16, tag=f"kproj{p}", name=f"kproj{p}") for p in range(4)]
    qproj = [proj.tile([P, SEQ], BF16, tag=f"qproj{p}", name=f"qproj{p}") for p in range(4)]
    v_ones = proj.tile([P, NKT, 8, 65], BF16, tag="v_ones")
    nc.vector.memset(v_ones[:], 1.0)

    def proj_kq_chunk(name, src_sb, dsts, blk, p):
        cs = slice(blk * 512, (blk + 1) * 512)
        ps = ps_sc.tile([P, 1024], F32, tag="scores")
        for dt in range(NDT):
            nc.tensor.matmul(
                ps[:, 0:512],
                w_sb[name][:, p, dt, :],
                src_sb[:, dt, cs],
                start=(dt == 0),
                stop=(dt == NDT - 1),
            )
        nc.vector.tensor_copy(dsts[p][:, cs], ps[:, 0:512])

    def proj_v_chunk(kb, sub):
        kt_i = kb * 4 + sub
        ps = ps_sc.tile([P, 1024], F32, tag="scores")
        for dt in range(NDT):
            nc.tensor.matmul(
                ps[:, 0:512],
                vt_sb[:, dt, kt_i * P:(kt_i + 1) * P],
                w_sb["wv"][:, dt, :],
                start=(dt == 0),
                stop=(dt == NDT - 1),
            )
        nc.vector.tensor_copy(
            v_ones[:, kt_i, :, 0:64],
            ps[:, 0:512].rearrange("p (h c) -> p h c", c=64),
        )

    # ---- phase 1: projections needed before the first pass --------------
    # (kt iteration order is rotated so blocks 2, 3, 0 can be projected
    # while the first pass runs over block 1's k-tiles)
    KT_ORDER = [4, 5, 6, 7, 8, 9, 10, 11, 12, 13, 14, 15, 0, 1, 2, 3]
    for p in range(4):
        proj_kq_chunk("wk", kt_sb, kproj, 1, p)
    for sub in range(4):
        proj_v_chunk(1, sub)
    for p in range(4):
        proj_kq_chunk("wq", qt_sb, qproj, 0, p)

    # ---- phase 2: attention --------------------------------------------
    # Per quad, 32 (kt, pi) steps are software-pipelined: scores(s)+exp(s)
    # are emitted two steps ahead of AV(s), so the PE never waits on the
    # ScalarE exp. Remaining K/V projection chunks are interleaved into
    # the first quad, and Q(qb+1) into each qb's second quad, keeping
    # both engines continuously fed.
    pending_tail = []

    for qb in range(NQB):
        qs = slice(qb * 512, (qb + 1) * 512)
        for pr in range(4):
            qb_, pr_ = qb, pr
            # fillers: (kind, blk, p) proj chunks emitted one list per step.
            # For the very first pass this covers everything beyond the two
            # upfront chunks; deadlines are met under the rotated KT_ORDER.
            filler = [[] for _ in range(16)]
            if qb == 0 and pr == 0:
                for j, kb in enumerate((2, 3, 0)):
                    for p in range(4):
                        filler[4 * j + p].append(("k", kb, p))
                        filler[4 * j + p].append(("v", kb, p))
            if pr == 3 and qb < NQB - 1:
                for p in range(4):
                    filler[4 * p + 1].append(("q", qb + 1, p))

            ot = [ps_ot.tile([P, 512], F32, tag="ot", name=f"ot{pr}_{hh}")
                  for hh in range(2)]
            e_tiles = {}

            def emit_scores(s):
                kt_i = KT_ORDER[s]
                sps = ps_sc.tile([P, 1024], F32, tag="scores")
                for hh in range(2):
                    rows = slice(64 * hh, 64 * (hh + 1))
                    nc.tensor.matmul(
                        sps[:, 512 * hh:512 * (hh + 1)],
                        kproj[pr][rows, kt_i * P:(kt_i + 1) * P],
                        qproj[pr][rows, qs],
                        start=True,
                        stop=True,
                        tile_position=(64 * hh, 0),
                    )
                e = epool.tile([P, 1024], BF16, tag="e")
                nc.scalar.activation(
                    e[:], sps[:], EXP,
                    bias=bmask_sb[:, kt_i:kt_i + 1], scale=0.125,
                )
                e_tiles[s] = e

            def emit_av(s):
                kt_i = KT_ORDER[s]
                e = e_tiles.pop(s)
                for hh in range(2):
                    h = 2 * pr + hh
                    nc.tensor.matmul(
                        ot[hh][0:65, :],
                        v_ones[:, kt_i, h, :],
                        e[:, 512 * hh:512 * (hh + 1)],
                        start=(s == 0),
                        stop=(s == NKT - 1),
                    )

            for s in range(16):
                emit_scores(s)
                if s == 2:
                    for t in pending_tail:
                        t()
                    pending_tail.clear()
                for kind, blk, p in filler[s]:
                    if kind == "k":
                        proj_kq_chunk("wk", kt_sb, kproj, blk, p)
                    elif kind == "v":
                        proj_v_chunk(blk, p)
                    else:
                        proj_kq_chunk("wq", qt_sb, qproj, blk, p)
                if s >= 2:
                    emit_av(s - 2)
            emit_av(14)
            emit_av(15)

            # ---- tail: copy out of PSUM now; transpose + normalize +
            # store deferred into the next pass so the PE queue is not
            # blocked on the copies at the pass boundary.
            ot_sbs = []
            for hh in range(2):
                ot_sb = opool.tile([P, 512], F32, tag="ot_sb")
                nc.vector.tensor_copy(ot_sb[0:65, :], ot[hh][0:65, :])
                ot_sbs.append(ot_sb)

            def tail_rest(qb=qb_, pr=pr_, ot_sbs=ot_sbs):
                o_part = oparts.tile([P, 4, 128], F32, tag="opart")
                for hh in range(2):
                    ot_sb = ot_sbs[hh]
                    tr = ps_sc.tile([P, 1024], F32, tag="scores",
                                    name=f"tr{qb}_{pr}_{hh}")
                    rcp = rpool.tile([P, 4], F32, tag="rcp")
                    for c in range(4):
                        nc.tensor.transpose(
                            tr[:, 65 * c:65 * c + 65],
                            ot_sb[0:65, c * P:(c + 1) * P],
                            ident_f[0:65, 0:65],
                        )
                    for c in range(4):
                        nc.vector.reciprocal(
                            rcp[:, c:c + 1], tr[:, 65 * c + 64:65 * c + 65]
                        )
                    for c in range(4):
                        nc.vector.tensor_scalar(
                            o_part[:, c, 64 * hh:64 * (hh + 1)],
                            tr[:, 65 * c:65 * c + 64],
                            rcp[:, c:c + 1],
                            None,
                            mybir.AluOpType.mult,
                        )
                nc.sync.dma_start(
                    out.ap()[qb * 512:(qb + 1) * 512, pr * 128:(pr + 1) * 128]
                    .rearrange("(c p) w -> p c w", p=P),
                    o_part[:],
                )

            pending_tail.append(tail_rest)

    for t in pending_tail:
        t()
    pending_tail.clear()


def build():
    global _compiled
    if _compiled is not None:
        return _compiled
    nc = bacc.Bacc("TRN2", target_bir_lowering=False, debug=False)
    qt = nc.dram_tensor("qt", [DM, SEQ], BF16, kind="ExternalInput")
    kt = nc.dram_tensor("kt", [DM, SEQ], BF16, kind="ExternalInput")
    vt = nc.dram_tensor("vt", [DM, SEQ], BF16, kind="ExternalInput")
    wq = nc.dram_tensor("wq", [DM, CPC], BF16, kind="ExternalInput")
    wk = nc.dram_tensor("wk", [DM, CPC], BF16, kind="ExternalInput")
    wv = nc.dram_tensor("wv", [DM, CPC], BF16, kind="ExternalInput")
    bmask = nc.dram_tensor("bmask", [P, NKT], F32, kind="ExternalInput")
    out = nc.dram_tensor("out", [SEQ, CPC], F32, kind="ExternalOutput")
    with tile.TileContext(nc) as tc:
        with ExitStack() as ctx:
            _emit(ctx, tc, qt, kt, vt, wq, wk, wv, bmask, out)
    nc.compile()
    _compiled = nc
    return nc


def make_in_maps(Q_seq, K_seq, V_seq, V_len, WQ, WK, WV):
    bf = ml_dtypes.bfloat16
    in_maps = []
    qkv_t = {}
    for b in range(B):
        qkv_t[b] = tuple(
            np.ascontiguousarray(x[b].T).astype(bf) for x in (Q_seq, K_seq, V_seq)
        )
    w_bf = {hg: tuple(
        np.ascontiguousarray(w[:, hg * CPC:(hg + 1) * CPC]).astype(bf)
        for w in (WQ, WK, WV)) for hg in range(2)}
    for core in range(NCORES):
        b, hg = divmod(core, 2)
        bm = np.zeros((P, NKT), np.float32)
        vl = int(V_len[b, 0])
        bm[vl % P, vl // P] = -1e6
        qt, kt, vt = qkv_t[b]
        wq, wk, wv = w_bf[hg]
        in_maps.append(
            {"qt": qt, "kt": kt, "vt": vt, "wq": wq, "wk": wk, "wv": wv,
             "bmask": bm}
        )
    return in_maps


def kernel(Q_seq, K_seq, V_seq, Q_len, V_len, WQ, WK, WV, _trace=False):
    nc = build()
    in_maps = make_in_maps(Q_seq, K_seq, V_seq, V_len, WQ, WK, WV)
    res = run_bass_kernel_spmd(
        nc, in_maps, core_ids=list(range(NCORES)), trace=_trace
    )
    out = np.empty((B, SEQ, H * DH), np.float32)
    for core in range(NCORES):
        b, hg = divmod(core, 2)
        out[b, :, hg * CPC:(hg + 1) * CPC] = res.results[core]["out"]
    for b in range(B):
        out[b, int(Q_len[b, 0]), :] = 0.0
    if _trace:
        kernel._last_results = res
    return out


# revision 38
# speedup vs baseline: 1.2557x; 1.0073x over previous
"""Multi-head attention (B=4, S=2048, D=1024, H=16, Dh=64) on 8 TRN2 NeuronCores.

Sharding: core = (batch, head_group) with 4 batches x 2 head-groups of 8 heads.
Fully data-parallel SPMD - no collectives. Each core computes
out[b, :, hg*512:(hg+1)*512].

v2: all matmul operands in bf16 (inputs cast host-side, halving DMA and
PE passes vs fp32r), and the softmax denominators come for free from a
ones-column fused into each head's V stationary ([128, 65] per head ->
output row 64 accumulates sum(exp)).

Per-core kernel:
  phase 1: project K^T (d-major), V (k-major + ones col), Q^T.
  phase 2: per q-block of 512 and head-quad of 4: scores S^T[k,q] per
           k-tile via row-paired Dh=64 matmuls, exp on ScalarE
           (scale=1/8, per-partition bias -1e6 at the masked key
           position), AV via [128,65] stationaries accumulating
           [O^T; sums] in per-head PSUM tiles, then PE-transpose back
           to q-major and normalize on VectorE.
"""

from contextlib import ExitStack

import ml_dtypes
import numpy as np

import concourse.bass as bass
import concourse.bacc as bacc
import concourse.mybir as mybir
import concourse.tile as tile
from concourse.bass_utils import run_bass_kernel_spmd
from concourse.masks import make_identity

B = 4
SEQ = 2048
DM = 1024
H = 16
DH = 64
NCORES = 8
CPC = 512          # output columns per core (8 heads x 64)
P = 128
NQB = SEQ // 512   # q blocks of 512
NKT = SEQ // P     # k tiles of 128
NDT = DM // P      # d_model tiles of 128

F32 = mybir.dt.float32
BF16 = mybir.dt.bfloat16
EXP = mybir.ActivationFunctionType.Exp

_compiled = None


def _emit(ctx: ExitStack, tc: tile.TileContext, qt, kt, vt, wq, wk, wv, bmask, out):
    nc = tc.nc

    small = ctx.enter_context(tc.tile_pool(name="small", bufs=1))
    in_sb = ctx.enter_context(tc.tile_pool(name="in_sb", bufs=1))
    proj = ctx.enter_context(tc.tile_pool(name="proj", bufs=1))
    wpool = ctx.enter_context(tc.tile_pool(name="wpool", bufs=1))
    epool = ctx.enter_context(tc.tile_pool(name="epool", bufs=8))
    opool = ctx.enter_context(tc.tile_pool(name="opool", bufs=3))
    rpool = ctx.enter_context(tc.tile_pool(name="rpool", bufs=4))
    oparts = ctx.enter_context(tc.tile_pool(name="oparts", bufs=3))
    ps_sc = ctx.enter_context(tc.tile_pool(name="ps_sc", bufs=3, space="PSUM"))
    ps_ot = ctx.enter_context(tc.tile_pool(name="ps_ot", bufs=2, space="PSUM"))

    ident_f = small.tile([P, P], F32)
    make_identity(nc, ident_f[:])
    ident = small.tile([P, P], BF16)
    nc.vector.tensor_copy(ident[:], ident_f[:])
    bmask_sb = small.tile([P, NKT], F32)
    nc.sync.dma_start(bmask_sb[:], bmask.ap())

    # wk/wq are laid out [p, pe-col-block, dt, 128] so each projection
    # chunk p only needs a quarter of the tile; wv stays dt-major (its
    # matmuls stream all 512 columns per dt).
    w_sb = {
        "wk": wpool.tile([P, 4, NDT, 128], BF16, tag="wk", name="wk"),
        "wq": wpool.tile([P, 4, NDT, 128], BF16, tag="wq", name="wq"),
        "wv": wpool.tile([P, NDT, CPC], BF16, tag="wv", name="wv"),
    }

    kt_sb = in_sb.tile([P, NDT, SEQ], BF16, tag="kt_sb")
    vt_sb = in_sb.tile([P, NDT, SEQ], BF16, tag="vt_sb")
    qt_sb = in_sb.tile([P, NDT, SEQ], BF16, tag="qt_sb")

    # DMA issue order follows first-use order: weights, then per-512-block
    # K, V, Q column slices.
    # DMA issue order matches first-use order: the first pass consumes
    # K/V block 1 (rotated kt order) and Q block 0, then projects blocks
    # 2, 3, 0 from inside the pass.
    def in_dma(t, t_sb, blk):
        cs = slice(blk * 512, (blk + 1) * 512)
        nc.sync.dma_start(
            t_sb[:, :, cs],
            t.ap().rearrange("(dt p) q -> p dt q", p=P)[:, :, cs],
        )

    def in_dma_dt(t, t_sb, blk):
        cs = slice(blk * 512, (blk + 1) * 512)
        src = t.ap().rearrange("(dt p) q -> p dt q", p=P)
        for dt in range(NDT):
            nc.sync.dma_start(
                t_sb[:, dt:dt + 1, cs], src[:, dt:dt + 1, cs]
            )

    def w_dma_g(name, w, g):
        nc.sync.dma_start(
            w_sb[name][:, g],
            w.ap().rearrange("(dt p) (g c) -> p g dt c", p=P, g=4)[:, g],
        )

    w_dma_g("wk", wk, 0)
    in_dma_dt(kt, kt_sb, 1)
    for g in range(1, 4):
        w_dma_g("wk", wk, g)
    nc.sync.dma_start(w_sb["wv"][:], wv.ap().rearrange("(dt p) c -> p dt c", p=P))
    in_dma_dt(vt, vt_sb, 1)
    w_dma_g("wq", wq, 0)
    in_dma_dt(qt, qt_sb, 0)
    for g in range(1, 4):
        w_dma_g("wq", wq, g)
    for blk in (2, 3, 0):
        in_dma(kt, kt_sb, blk)
        in_dma(vt, vt_sb, blk)
    for blk in (1, 2, 3):
        in_dma(qt, qt_sb, blk)

    kproj = [proj.tile([P, SEQ], BF16, tag=f"kproj{p}", name=f"kproj{p}") for p in range(4)]
    qproj = [proj.tile([P, SEQ], BF16, tag=f"qproj{p}", name=f"qproj{p}") for p in range(4)]
    v_ones = proj.tile([P, NKT, 8, 65], BF16, tag="v_ones")
    nc.vector.memset(v_ones[:], 1.0)

    def proj_kq_chunk(name, src_sb, dsts, blk, p):
        cs = slice(blk * 512, (blk + 1) * 512)
        ps = ps_sc.tile([P, 1024], F32, tag="scores")
        for dt in range(NDT):
            nc.tensor.matmul(
                ps[:, 0:512],
                w_sb[name][:, p, dt, :],
                src_sb[:, dt, cs],
                start=(dt == 0),
                stop=(dt == NDT - 1),
            )
        nc.vector.tensor_copy(dsts[p][:, cs], ps[:, 0:512])

    def proj_v_chunk(kb, sub):
        kt_i = kb * 4 + sub
        ps = ps_sc.tile([P, 1024], F32, tag="scores")
        for dt in range(NDT):
            nc.tensor.matmul(
                ps[:, 0:512],
                vt_sb[:, dt, kt_i * P:(kt_i + 1) * P],
                w_sb["wv"][:, dt, :],
                start=(dt == 0),
                stop=(dt == NDT - 1),
            )
        nc.vector.tensor_copy(
            v_ones[:, kt_i, :, 0:64],
            ps[:, 0:512].rearrange("p (h c) -> p h c", c=64),
        )

    # ---- phase 1: projections needed before the first pass --------------
    # (kt iteration order is rotated so blocks 2, 3, 0 can be projected
    # while the first pass runs over block 1's k-tiles)
    KT_ORDER = [4, 5, 6, 7, 8, 9, 10, 11, 12, 13, 14, 15, 0, 1, 2, 3]
    for p in range(4):
        proj_kq_chunk("wk", kt_sb, kproj, 1, p)
    for sub in range(4):
        proj_v_chunk(1, sub)
    for p in range(4):
        proj_kq_chunk("wq", qt_sb, qproj, 0, p)

    # ---- phase 2: attention --------------------------------------------
    # Per quad, 32 (kt, pi) steps are software-pipelined: scores(s)+exp(s)
    # are emitted two steps ahead of AV(s), so the PE never waits on the
    # ScalarE exp. Remaining K/V projection chunks are interleaved into
    # the first quad, and Q(qb+1) into each qb's second quad, keeping
    # both engines continuously fed.
    pending_tail = []

    for qb in range(NQB):
        qs = slice(qb * 512, (qb + 1) * 512)
        for pr in range(4):
            qb_, pr_ = qb, pr
            # fillers: (kind, blk, p) proj chunks emitted one list per step.
            # For the very first pass this covers everything beyond the two
            # upfront chunks; deadlines are met under the rotated KT_ORDER.
            filler = [[] for _ in range(16)]
            if qb == 0:
                # pass pr only consumes kproj[pr], so K chunks for blocks
                # 2/3/0 are spread across passes; V chunks must all land
                # in pass 0 (every pass's AV reads v_ones).
                for j, kb in enumerate((2, 3, 0)):
                    filler[4 * j].append(("k", kb, pr))
                    if pr == 0:
                        for p in range(4):
                            filler[4 * j + p].append(("v", kb, p))
            if pr == 3 and qb < NQB - 1:
                for p in range(4):
                    filler[4 * p + 1].append(("q", qb + 1, p))

            ot = [ps_ot.tile([P, 512], F32, tag="ot", name=f"ot{pr}_{hh}")
                  for hh in range(2)]
            e_tiles = {}

            def emit_scores(s):
                kt_i = KT_ORDER[s]
                sps = ps_sc.tile([P, 1024], F32, tag="scores")
                for hh in range(2):
                    rows = slice(64 * hh, 64 * (hh + 1))
                    nc.tensor.matmul(
                        sps[:, 512 * hh:512 * (hh + 1)],
                        kproj[pr][rows, kt_i * P:(kt_i + 1) * P],
                        qproj[pr][rows, qs],
                        start=True,
                        stop=True,
                        tile_position=(64 * hh, 0),
                    )
                e = epool.tile([P, 1024], BF16, tag="e")
                nc.scalar.activation(
                    e[:], sps[:], EXP,
                    bias=bmask_sb[:, kt_i:kt_i + 1], scale=0.125,
                )
                e_tiles[s] = e

            def emit_av(s):
                kt_i = KT_ORDER[s]
                e = e_tiles.pop(s)
                for hh in range(2):
                    h = 2 * pr + hh
                    nc.tensor.matmul(
                        ot[hh][0:65, :],
                        v_ones[:, kt_i, h, :],
                        e[:, 512 * hh:512 * (hh + 1)],
                        start=(s == 0),
                        stop=(s == NKT - 1),
                    )

            for s in range(16):
                emit_scores(s)
                if s == 2:
                    for t in pending_tail:
                        t()
                    pending_tail.clear()
                for kind, blk, p in filler[s]:
                    if kind == "k":
                        proj_kq_chunk("wk", kt_sb, kproj, blk, p)
                    elif kind == "v":
                        proj_v_chunk(blk, p)
                    else:
                        proj_kq_chunk("wq", qt_sb, qproj, blk, p)
                if s >= 2:
                    emit_av(s - 2)
            emit_av(14)
            emit_av(15)

            # ---- tail: copy out of PSUM now; transpose + normalize +
            # store deferred into the next pass so the PE queue is not
            # blocked on the copies at the pass boundary.
            ot_sbs = []
            for hh in range(2):
                ot_sb = opool.tile([P, 512], F32, tag="ot_sb")
                nc.vector.tensor_copy(ot_sb[0:65, :], ot[hh][0:65, :])
                ot_sbs.append(ot_sb)

            def tail_rest(qb=qb_, pr=pr_, ot_sbs=ot_sbs):
                o_part = oparts.tile([P, 4, 128], F32, tag="opart")
                for hh in range(2):
                    ot_sb = ot_sbs[hh]
                    tr = ps_sc.tile([P, 1024], F32, tag="scores",
                                    name=f"tr{qb}_{pr}_{hh}")
                    rcp = rpool.tile([P, 4], F32, tag="rcp")
                    for c in range(4):
                        nc.tensor.transpose(
                            tr[:, 65 * c:65 * c + 65],
                            ot_sb[0:65, c * P:(c + 1) * P],
                            ident_f[0:65, 0:65],
                        )
                    for c in range(4):
                        nc.vector.reciprocal(
                            rcp[:, c:c + 1], tr[:, 65 * c + 64:65 * c + 65]
                        )
                    for c in range(4):
                        nc.vector.tensor_scalar(
                            o_part[:, c, 64 * hh:64 * (hh + 1)],
                            tr[:, 65 * c:65 * c + 64],
                            rcp[:, c:c + 1],
                            None,
                            mybir.AluOpType.mult,
                        )
                nc.sync.dma_start(
                    out.ap()[qb * 512:(qb + 1) * 512, pr * 128:(pr + 1) * 128]
                    .rearrange("(c p) w -> p c w", p=P),
                    o_part[:],
                )

            pending_tail.append(tail_rest)

    for t in pending_tail:
        t()
    pending_tail.clear()


def build():
    global _compiled
    if _compiled is not None:
        return _compiled
    nc = bacc.Bacc("TRN2", target_bir_lowering=False, debug=False)
    qt = nc.dram_tensor("qt", [DM, SEQ], BF16, kind="ExternalInput")
    kt = nc.dram_tensor("kt", [DM, SEQ], BF16, kind="ExternalInput")
    vt = nc.dram_tensor("vt", [DM, SEQ], BF16, kind="ExternalInput")
    wq = nc.dram_tensor("wq", [DM, CPC], BF16, kind="ExternalInput")
    wk = nc.dram_tensor("wk", [DM, CPC], BF16, kind="ExternalInput")
    wv = nc.dram_tensor("wv", [DM, CPC], BF16, kind="ExternalInput")
    bmask = nc.dram_tensor("bmask", [P, NKT], F32, kind="ExternalInput")
    out = nc.dram_tensor("out", [SEQ, CPC], F32, kind="ExternalOutput")
    with tile.TileContext(nc) as tc:
        with ExitStack() as ctx:
            _emit(ctx, tc, qt, kt, vt, wq, wk, wv, bmask, out)
    nc.compile()
    _compiled = nc
    return nc


def make_in_maps(Q_seq, K_seq, V_seq, V_len, WQ, WK, WV):
    bf = ml_dtypes.bfloat16
    in_maps = []
    qkv_t = {}
    for b in range(B):
        qkv_t[b] = tuple(
            np.ascontiguousarray(x[b].T).astype(bf) for x in (Q_seq, K_seq, V_seq)
        )
    w_bf = {hg: tuple(
        np.ascontiguousarray(w[:, hg * CPC:(hg + 1) * CPC]).astype(bf)
        for w in (WQ, WK, WV)) for hg in range(2)}
    for core in range(NCORES):
        b, hg = divmod(core, 2)
        bm = np.zeros((P, NKT), np.float32)
        vl = int(V_len[b, 0])
        bm[vl % P, vl // P] = -1e6
        qt, kt, vt = qkv_t[b]
        wq, wk, wv = w_bf[hg]
        in_maps.append(
            {"qt": qt, "kt": kt, "vt": vt, "wq": wq, "wk": wk, "wv": wv,
             "bmask": bm}
        )
    return in_maps


def kernel(Q_seq, K_seq, V_seq, Q_len, V_len, WQ, WK, WV, _trace=False):
    nc = build()
    in_maps = make_in_maps(Q_seq, K_seq, V_seq, V_len, WQ, WK, WV)
    res = run_bass_kernel_spmd(
        nc, in_maps, core_ids=list(range(NCORES)), trace=_trace
    )
    out = np.empty((B, SEQ, H * DH), np.float32)
    for core in range(NCORES):
        b, hg = divmod(core, 2)
        out[b, :, hg * CPC:(hg + 1) * CPC] = res.results[core]["out"]
    for b in range(B):
        out[b, int(Q_len[b, 0]), :] = 0.0
    if _trace:
        kernel._last_results = res
    return out
